# revision 84
# baseline (speedup 1.0000x reference)
"""DiffusionTransformerLayer on 8 Trainium2 NeuronCores — collective-free,
fp8 DoubleRow edition.

Sharding: rows (B*N = 2048 tokens) split 256/core for everything except k/v,
which each core (re)computes for its batch's FULL 1024 rows locally — the
duplicated PE work replaces the AllGather entirely (no cross-core sync, no
DRAM staging round-trip, no collective latency).

Per-core inputs arrive ROTATED so the core's own 256 rows are rows 0-255 of
the full-batch tensors; the pair-bias exp(z^T) is permuted to match, so one
SPMD graph serves all cores.

v2: every GEMM with a 256+-elem contraction runs in fp8e4m3 DoubleRow mode
(2 k-tiles of 128 contracted per instruction at 0.5 PE cycles/row = 4x bf16
throughput). Weights are host-scaled by 16 into fp8's sweet spot and the
1/16 descale rides existing PSUM readouts (ACT activation scale= / DVE
tensor_scalar mult). Activation operands (snT/bT/sT/a2T/p/v/xT/hT) are
written as fp8 by the PSUM->SBUF copies that existed anyway. Numerics: the
attention block contributes <=0.017 abs to the output (tolerance ~0.1 abs)
and the transition <=0.13, so fp8's ~6% relative error is far inside
budget. Scores stay bf16 (48-elem contraction, no DR possible).

Layout strategy (as v1):
  - full-batch AdaLN1 runs feature-on-partition (transposed): snT/anT via PE
    transposes; b^T assembled in the orientation the k/v/q/g projections eat.
  - attention: scores transposed (ST[k,q]), softmax over k = exp on ACT *
    host-precomputed exp(z^T) (Pool, fp8), row-sum via memset ones-columns
    in v, grouped reciprocal+broadcast via indicator matmul.
  - fp32 PSUM accumulation throughout.

Inputs ride in TWO flat blobs (bf16 + fp8): single-digit PJRT buffer count
keeps dispatch cost down.
"""
import os

import numpy as np
import ml_dtypes

import concourse.bacc as bacc
import concourse.tile as tile
from concourse import mybir
from concourse.bass_utils import run_bass_kernel_spmd

F32 = mybir.dt.float32
BF16 = mybir.dt.bfloat16
F8 = mybir.dt.float8e4
AF = mybir.ActivationFunctionType
OP = mybir.AluOpType
DR = mybir.MatmulPerfMode.DoubleRow
NPBF16 = ml_dtypes.bfloat16
NPF8 = ml_dtypes.float8_e4m3

B, N, D, H = 2, 1024, 768, 16
HD, HDP = 48, 64
HP = H * HDP          # 1024 padded head dims
HID = 1536
R = 256               # own rows per core
NF = 1024             # full-batch rows per core
FC = D // 128         # 6 feature chunks
KP = FC // 2          # 3 contraction k-pairs for 768
RT = NF // 128        # 8 full-batch row tiles
EPS = 1e-5
SCALE = HD ** -0.5
WSC = 16.0            # host-side fp8 weight scale
IW = 1.0 / WSC
IW2 = 1.0 / (WSC * WSC)

_LAYOUT = [               # bf16 blob
    ("s_full", (NF, D)),
    ("a_full", (NF, D)),
    ("ident_in", (128, 128)),
    ("ind2_in", (128, 128)),
    ("qb_in", (8, 128)),
    ("ssb1_in", (6, 128)),
    ("ssb2_in", (6, 128)),
    ("bop1_in", (1, D)),
    ("bop2_in", (1, D)),
]
_OFFSETS = {}
_TOTAL = 0
for _nm, _sh in _LAYOUT:
    _OFFSETS[_nm] = _TOTAL
    _TOTAL += int(np.prod(_sh))

_LAYOUT8 = [              # fp8 blob
    ("ez_in", (8, 128, 2, 8, R)),     # [pair, kpart, hb, kchunk, q]
    ("w_ss1", (D, D)),
    ("w_sb1", (D, D)),
    ("w_k", (D, HP)),
    ("w_v", (D, HP)),
    ("w_q", (D, HP)),
    ("w_g", (D, HP)),
    ("w_ow", (HP, D)),
    ("w_ss2", (D, D)),
    ("w_sb2", (D, D)),
    ("w_op1", (D, D)),
    ("w_op2", (D, D)),
    ("w_swu", (D, HID)),
    ("w_swg", (D, HID)),
    ("w_ab", (D, HID)),
    ("w_ba", (HID, D)),
    ("ident8_in", (128, 128)),
]
_OFFSETS8 = {}
_TOTAL8 = 0
for _nm, _sh in _LAYOUT8:
    _OFFSETS8[_nm] = _TOTAL8
    _TOTAL8 += int(np.prod(_sh))

_NC_CACHE = {}


def _build_nc():
    stage = int(os.environ.get("KSTAGE", "4"))
    nc = bacc.Bacc("TRN2", target_bir_lowering=False, debug=False, num_devices=8)

    xin = nc.dram_tensor("xin", [_TOTAL], BF16, kind="ExternalInput")
    w8in = nc.dram_tensor("w8in", [_TOTAL8], F8, kind="ExternalInput")
    y_out = nc.dram_tensor("y", [R, D], F32, kind="ExternalOutput")

    def v2d(name):
        shp = dict(_LAYOUT)[name]
        rows, cols = shp[0], int(np.prod(shp[1:]))
        off = _OFFSETS[name]
        return xin.ap()[off:off + rows * cols].rearrange("(r c) -> r c", c=cols)

    def vez(t):
        off = _OFFSETS8["ez_in"] + t * (128 * 2 * 8 * R)
        return w8in.ap()[off:off + 128 * 2 * 8 * R].rearrange(
            "(p a b c) -> p a b c", a=2, b=8, c=R)

    with tile.TileContext(nc) as tc:
        from contextlib import ExitStack
        with ExitStack() as ctx:
            cst = ctx.enter_context(tc.tile_pool(name="cst", bufs=1))
            nat = ctx.enter_context(tc.tile_pool(name="nat", bufs=1))
            tp = ctx.enter_context(tc.tile_pool(name="tp", bufs=1))
            kvp = ctx.enter_context(tc.tile_pool(name="kvp", bufs=1))
            wp = ctx.enter_context(tc.tile_pool(name="wp", bufs=3))
            ezp = ctx.enter_context(tc.tile_pool(name="ezp", bufs=2))
            pp = ctx.enter_context(tc.tile_pool(name="pp", bufs=4))
            own = ctx.enter_context(tc.tile_pool(name="own", bufs=1))
            lnp = ctx.enter_context(tc.tile_pool(name="lnp", bufs=3))
            tmp = ctx.enter_context(tc.tile_pool(name="tmp", bufs=2))
            psb = ctx.enter_context(tc.tile_pool(name="psb", bufs=2, space="PSUM"))
            pss = ctx.enter_context(tc.tile_pool(name="pss", bufs=4, space="PSUM"))

            def load_w8(wname, nchunks, name, eng=None):
                """One-trigger fp8 weight load as [128, nchunks, width]."""
                shp = dict(_LAYOUT8)[wname]
                wt = wp.tile([128, nchunks, shp[1]], F8, tag="wslot",
                             bufs=5, name=name)
                v = w8in.ap()[_OFFSETS8[wname]:
                              _OFFSETS8[wname] + 128 * nchunks * shp[1]]
                v = v.rearrange("(c p f) -> p c f", p=128, f=shp[1])
                (eng or nc.sync).dma_start(out=wt, in_=v)
                return wt

            for _rep in range(int(os.environ.get("KREP", "1"))):
                # ---------------- bulk loads (SP) ----------------
                ident = cst.tile([128, 128], BF16)
                nc.sync.dma_start(out=ident, in_=v2d("ident_in"))
                eps_t = cst.tile([128, 1], F32)
                nc.vector.memset(eps_t, EPS)
                # s/a loads spread across queues: all 16 tiles resident in
                # ~3us instead of ~10us serial on SP
                s_sb = []
                for rt in range(RT):
                    st = nat.tile([128, D], BF16, tag="nat768", bufs=18,
                                  name=f"s{rt}")
                    e = nc.sync if rt % 2 == 0 else nc.gpsimd
                    e.dma_start(
                        out=st, in_=v2d("s_full")[128 * rt:128 * (rt + 1), :])
                    s_sb.append(st)
                a_sb = []
                for rt in range(RT):
                    at = nat.tile([128, D], BF16, tag="nat768", bufs=18,
                                  name=f"a{rt}")
                    e = nc.gpsimd if rt % 2 == 0 else nc.sync
                    e.dma_start(
                        out=at, in_=v2d("a_full")[128 * rt:128 * (rt + 1), :])
                    a_sb.append(at)
                # op-gate weights early: their matmuls fill the LN-bound
                # front. Pool queue, not ACT (ACT holds for the transfer).
                wop1 = load_w8("w_op1", FC, "w_op1", eng=nc.gpsimd)
                wop2 = load_w8("w_op2", FC, "w_op2", eng=nc.gpsimd)
                w1 = load_w8("w_ss1", FC, "w_ss1", eng=nc.gpsimd)
                w2 = load_w8("w_sb1", FC, "w_sb1", eng=nc.gpsimd)
                bop1 = wp.tile([1, D], BF16, tag="wbias", bufs=2, name="b_op1")
                nc.sync.dma_start(out=bop1, in_=v2d("bop1_in"))
                bop2 = wp.tile([1, D], BF16, tag="wbias", bufs=2, name="b_op2")
                nc.sync.dma_start(out=bop2, in_=v2d("bop2_in"))
                a_own = []
                for rt in range(2):
                    at = own.tile([128, D], BF16, tag="aown", bufs=2,
                                  name=f"ao{rt}")
                    nc.sync.dma_start(
                        out=at, in_=v2d("a_full")[128 * rt:128 * (rt + 1), :])
                    a_own.append(at)

                # ---------------- constants (SP) ----------------
                ind2 = cst.tile([128, 128], BF16)
                nc.sync.dma_start(out=ind2, in_=v2d("ind2_in"))
                ident8 = cst.tile([128, 128], F8)
                nc.sync.dma_start(
                    out=ident8,
                    in_=w8in.ap()[_OFFSETS8["ident8_in"]:
                                  _OFFSETS8["ident8_in"] + 128 * 128]
                    .rearrange("(r c) -> r c", c=128))
                rb = cst.tile([128, 256], BF16)
                nc.vector.memset(rb, 0.0)
                qb_bf = cst.tile([128, 8], BF16)
                nc.sync.dma_start(out=qb_bf,
                                  in_=v2d("qb_in").rearrange("t p -> p t"))
                qb = cst.tile([128, 8], F32)
                nc.gpsimd.tensor_copy(out=qb, in_=qb_bf)
                ssb1 = cst.tile([128, 6], BF16)
                nc.sync.dma_start(out=ssb1,
                                  in_=v2d("ssb1_in").rearrange("b p -> p b"))
                ssb1f = cst.tile([128, 6], F32)
                nc.gpsimd.tensor_copy(out=ssb1f, in_=ssb1)
                ssb2 = cst.tile([128, 6], BF16)
                nc.sync.dma_start(out=ssb2,
                                  in_=v2d("ssb2_in").rearrange("b p -> p b"))
                ssb2f = cst.tile([128, 6], F32)
                nc.gpsimd.tensor_copy(out=ssb2f, in_=ssb2)
                ones1 = cst.tile([1, 256], BF16)
                nc.vector.memset(ones1, 1.0)

                # ---------------- helpers ----------------
                def ln_stats(src, mvt, j):
                    """bn stats of one [128, D] tile into mvt[:, j, :].
                    Two 384-wide subgroups (<=512 hw max): fewer instrs."""
                    stats = lnp.tile([128, 2, 6], F32, tag="lnstats")
                    s3 = src.rearrange("p (s c) -> p s c", s=2)
                    for sg in range(2):
                        nc.vector.bn_stats(out=stats[:, sg, :],
                                           in_=s3[:, sg, :])
                    nc.vector.bn_aggr(out=mvt[:, j, :], in_=stats)

                def ln_rstd(mvt, n, tag):
                    """ONE batched sqrt + reciprocal for n (mean,var) pairs
                    (act-table swaps cost 1.3us each; never alternate)."""
                    rstd = lnp.tile([128, n], F32, tag=tag, bufs=1)
                    nc.scalar.activation(
                        out=rstd,
                        in_=mvt[:, :, 1:2].rearrange("p a b -> p (a b)"),
                        func=AF.Sqrt, bias=eps_t, scale=1.0)
                    nc.vector.reciprocal(out=rstd, in_=rstd)
                    return rstd

                def ln_apply(src, dst, mvt, rstd, j, eng=None):
                    (eng or nc.vector).tensor_scalar(
                        out=dst, in0=src,
                        scalar1=mvt[:, j, 0:1],
                        scalar2=rstd[:, j:j + 1],
                        op0=OP.subtract, op1=OP.mult)

                def ln_negmr(mvt, rstd, n):
                    """-mean*rstd per group: bias operand for ACT-side LN
                    applies (Copy is in every act table set: swap-free)."""
                    negmr = lnp.tile([128, n], F32, tag=f"negmr{n}", bufs=1)
                    nc.vector.tensor_tensor(
                        out=negmr, in0=mvt[:, :, 0], in1=rstd,
                        op=OP.mult)
                    nc.vector.tensor_scalar(out=negmr, in0=negmr,
                                            scalar1=-1.0, scalar2=None,
                                            op0=OP.mult)
                    return negmr

                def ln_apply_act(src, dst, rstd, negmr, j):
                    nc.scalar.activation(out=dst, in_=src, func=AF.Identity,
                                         bias=negmr[:, j:j + 1],
                                         scale=rstd[:, j:j + 1])

                def ln_pair(sa, sb_, na, nb, dsts=None, apply_eng=None):
                    """LN of two tiles (own stats+sqrt; used off the front)."""
                    mvs = lnp.tile([128, 2, 2], F32, tag="lnmv")
                    for j, s in enumerate((sa, sb_)):
                        ln_stats(s, mvs, j)
                    rstd = lnp.tile([128, 2], F32, tag="lnrstd")
                    nc.scalar.activation(
                        out=rstd,
                        in_=mvs[:, :, 1:2].rearrange("p a b -> p (a b)"),
                        func=AF.Sqrt, bias=eps_t, scale=1.0)
                    nc.vector.reciprocal(out=rstd, in_=rstd)
                    outs = []
                    for j, (s, nm) in enumerate(((sa, na), (sb_, nb))):
                        if dsts is None:
                            d = nat.tile([128, D], BF16, tag="nat768", bufs=18,
                                         name=nm)
                        else:
                            d = dsts[j]
                        ln_apply(s, d, mvs, rstd, j, apply_eng)
                        outs.append(d)
                    return outs

                def transpose_group(dst3d, srcs, rt0, i0=0):
                    """Transpose len(srcs) bf16 row-tiles into fp8
                    dst3d[:, fc, cols]; PSUM-read casts on DVE/ACT."""
                    g = len(srcs)
                    for fc in range(FC):
                        pst = pss.tile([128, 128 * g], BF16, tag="pss",
                                       name="pst")
                        for j, s in enumerate(srcs):
                            nc.tensor.transpose(
                                out=pst[:, 128 * j:128 * (j + 1)],
                                in_=s[:, 128 * fc:128 * (fc + 1)],
                                identity=ident)
                        dslice = dst3d[:, fc, 128 * rt0:128 * (rt0 + g)]
                        if (i0 + fc) % 2 == 0:
                            nc.vector.tensor_copy(out=dslice, in_=pst)
                        else:
                            nc.scalar.copy(out=dslice, in_=pst)


                def proj_own_dr(lhs3d, w3d, out_cols, bias=None):
                    """Own-row DR projection -> 2 PSUM tiles [128, out_cols].
                    lhs3d [128, FC, >=256] fp8 acts; w3d [128, FC, out_cols]."""
                    pso = [psb.tile([128, out_cols], F32, tag="psb",
                                    name="ps_nat") for _ in range(2)]
                    ncol = [(c, min(c + 512, out_cols))
                            for c in range(0, out_cols, 512)]
                    for kp in range(KP):
                        for rt in range(2):
                            for (cs, ce) in ncol:
                                nc.tensor.matmul(
                                    out=pso[rt][:, cs:ce],
                                    lhsT=lhs3d[:, 2 * kp:2 * kp + 2,
                                               128 * rt:128 * (rt + 1)],
                                    rhs=w3d[:, 2 * kp:2 * kp + 2, cs:ce],
                                    start=(kp == 0), stop=False,
                                    perf_mode=DR)
                    for rt in range(2):
                        for (cs, ce) in ncol:
                            nc.tensor.matmul(
                                out=pso[rt][:, cs:ce],
                                lhsT=ones1[:, 128 * rt:128 * rt + 128],
                                rhs=bias[:, cs:ce],
                                start=False, stop=True)
                    return pso

                # ------------- front: sT, LN(s) + snT, op gates -------------
                sT = own.tile([128, FC, 256], F8, tag="sT", name="sT")
                transpose_group(sT, s_sb[:2], 0)

                snT = tp.tile([128, FC, NF], F8, tag="snT", name="snT")
                # all 8 stats -> ONE sqrt (act-table swaps cost 1.3us each)
                mvs_s = lnp.tile([128, 8, 2], F32, tag="lnmv8s", bufs=1)
                for rt in range(RT):
                    ln_stats(s_sb[rt], mvs_s, rt)
                rstd_s = ln_rstd(mvs_s, 8, "lnrs")
                negmr_s = ln_negmr(mvs_s, rstd_s, 8)
                sn = []
                ps_og = ps_opg = None
                for pr in range(4):
                    for j in (0, 1):
                        i = 2 * pr + j
                        d = nat.tile([128, D], BF16, tag="nat768", bufs=18,
                                     name=f"sn{i}")
                        if i % 4 == 3:
                            ln_apply_act(s_sb[i], d, rstd_s, negmr_s, i)
                        else:
                            ln_apply(s_sb[i], d, mvs_s, rstd_s, i,
                                     eng=(nc.vector if i % 4 == 0
                                          else nc.gpsimd))
                        sn.append(d)
                    if pr % 2 == 1:
                        transpose_group(snT, sn[2 * pr - 2:2 * pr + 2],
                                         2 * pr - 2, i0=pr * FC)
                    if pr == 1:
                        ps_og = proj_own_dr(sT, wop1, D, bias=bop1)
                    if pr == 2:
                        ps_opg = proj_own_dr(sT, wop2, D, bias=bop2)

                og_sb = [own.tile([128, D], BF16, tag=f"og{rt}", name=f"og{rt}")
                         for rt in range(2)]
                opg_sb = [own.tile([128, D], BF16, tag=f"opg{rt}",
                                   name=f"opg{rt}") for rt in range(2)]
                for rt in range(2):
                    nc.scalar.activation(out=og_sb[rt], in_=ps_og[rt],
                                         func=AF.Sigmoid, scale=IW)
                for rt in range(2):
                    nc.scalar.activation(out=opg_sb[rt], in_=ps_opg[rt],
                                         func=AF.Sigmoid, scale=IW)

                # ------------- LN(a): stats now; batched sqrt later ---------
                # a-stats OFF the DVE (which is busy with s-stats): sums via
                # ACT (Identity/Square + accum_out: both live in every act
                # table, swap-free) and Pool (tensor ops + accum_out), then
                # var = E[x^2] - E[x]^2 (no cancellation risk: x ~ N(0,1)).
                mvs8 = lnp.tile([128, 8, 2], F32, tag="lnmv8", bufs=1)
                sx8 = lnp.tile([128, 8], F32, tag="sx8", bufs=1)
                sq8 = lnp.tile([128, 8], F32, tag="sq8", bufs=1)
                for rt in range(RT):
                    if rt < 6:
                        ln_stats(a_sb[rt], mvs8, rt)
                        continue
                    scr = tmp.tile([128, D], F32, tag="scr", bufs=2)
                    scr2 = tmp.tile([128, D], F32, tag="scr", bufs=2)
                    nc.scalar.activation(out=scr, in_=a_sb[rt],
                                         func=AF.Identity,
                                         accum_out=sx8[:, rt:rt + 1])
                    nc.scalar.activation(out=scr2, in_=a_sb[rt],
                                         func=AF.Square,
                                         accum_out=sq8[:, rt:rt + 1])
                nc.vector.tensor_scalar(out=mvs8[:, 6:, 0], in0=sx8[:, 6:],
                                        scalar1=1.0 / D, scalar2=None,
                                        op0=OP.mult)
                nc.vector.tensor_scalar(out=mvs8[:, 6:, 1], in0=sq8[:, 6:],
                                        scalar1=1.0 / D, scalar2=None,
                                        op0=OP.mult)
                msq = lnp.tile([128, 8], F32, tag="msq", bufs=1)
                nc.vector.tensor_mul(msq[:, 6:], mvs8[:, 6:, 0],
                                     mvs8[:, 6:, 0])
                nc.vector.tensor_sub(mvs8[:, 6:, 1], mvs8[:, 6:, 1],
                                     msq[:, 6:])

                wk = load_w8("w_k", FC, "wk")

                rstd8 = ln_rstd(mvs8, 8, "lnra")
                # gates: sigmoid(ps/16); the 1/256 double-fp8 descale of
                # ps_x / ps_t is fused into their scalar_tensor_tensor muls.
                negmr_a = ln_negmr(mvs8, rstd8, 8)
                an = []
                for rt in range(RT):
                    d = nat.tile([128, D], BF16, tag="nat768", bufs=18,
                                 name=f"an{rt}")
                    if rt % 4 == 3:
                        ln_apply_act(a_sb[rt], d, rstd8, negmr_a, rt)
                    else:
                        ln_apply(a_sb[rt], d, mvs8, rstd8, rt,
                                 eng=(nc.vector if rt % 4 == 0
                                      else nc.gpsimd))
                    an.append(d)

                # ------------- AdaLN1, transposed, fp8 DR -------------------
                sig1T = tp.tile([128, FC, NF], F8, tag="sig1T", name="sig1T")
                for ob in range(FC):
                    ps = psb.tile([128, NF], F32, tag="psb", name="ps_ss1")
                    for kp in range(KP):
                        for cs in (0, 512):
                            nc.tensor.matmul(
                                out=ps[:, cs:cs + 512],
                                lhsT=w1[:, 2 * kp:2 * kp + 2,
                                        128 * ob:128 * (ob + 1)],
                                rhs=snT[:, 2 * kp:2 * kp + 2, cs:cs + 512],
                                start=(kp == 0), stop=(kp == KP - 1),
                                perf_mode=DR)
                    nc.scalar.activation(
                        out=sig1T[:, ob, :], in_=ps, func=AF.Sigmoid,
                        bias=ssb1f[:, ob:ob + 1], scale=IW)

                # anT transposes must precede sb1T psum recycling
                anT = tp.tile([128, FC, NF], F8, tag="anT", name="anT")
                for rtg in (0, 4):
                    transpose_group(anT, an[rtg:rtg + 4], rtg, i0=rtg)

                # w_sb1 is fp8 at TRUE scale (not x16): its PSUM needs no
                # descale, so the add reads PSUM directly — no staging copy.
                bT = tp.tile([128, FC, NF], F8, tag="bT", name="bT")
                for ob in range(FC):
                    ps = psb.tile([128, NF], F32, tag="psb", name="ps_sb1")
                    for kp in range(KP):
                        for cs in (0, 512):
                            nc.tensor.matmul(
                                out=ps[:, cs:cs + 512],
                                lhsT=w2[:, 2 * kp:2 * kp + 2,
                                        128 * ob:128 * (ob + 1)],
                                rhs=snT[:, 2 * kp:2 * kp + 2, cs:cs + 512],
                                start=(kp == 0), stop=(kp == KP - 1),
                                perf_mode=DR)
                    nc.gpsimd.tensor_mul(bT[:, ob, :], anT[:, ob, :],
                                         sig1T[:, ob, :])
                    nc.vector.tensor_add(bT[:, ob, :], bT[:, ob, :], ps)

                if stage == 1:
                    for rt in range(2):
                        yt = tmp.tile([128, D], F32, tag="yt", bufs=2)
                        nc.vector.tensor_copy(out=yt, in_=sn[rt])
                        nc.sync.dma_start(
                            out=y_out.ap()[128 * rt:128 * (rt + 1), :], in_=yt)

                if stage >= 2:
                    # ---------------- k/q/v/g projections ------------------
                    wv = load_w8("w_v", FC, "wv")
                    wq = load_w8("w_q", FC, "wq")
                    # fp8 k/q with a ZERO second ktile plane: DoubleRow sums
                    # ktile0 (real) + ktile1 (zeros) so the 48-contraction
                    # score matmuls run at 0.5 cycles/row instead of 1.0
                    kt4d = kvp.tile([128, 8, 2, NF], F8, tag="kt4d",
                                    name="kt4d")
                    nc.gpsimd.memset(kt4d[:, :, 1, :], 0.0)
                    qt4d = own.tile([128, 8, 2, 256], F8, tag="qt4d",
                                    name="qt4d")
                    nc.gpsimd.memset(qt4d[:, :, 1, :], 0.0)

                    def emit_kt(t):
                        ps = psb.tile([128, NF], F32, tag="psb", name="ps_kt")
                        for kp in range(KP):
                            for cs in (0, 512):
                                nc.tensor.matmul(
                                    out=ps[:, cs:cs + 512],
                                    lhsT=wk[:, 2 * kp:2 * kp + 2,
                                            128 * t:128 * (t + 1)],
                                    rhs=bT[:, 2 * kp:2 * kp + 2, cs:cs + 512],
                                    start=(kp == 0), stop=(kp == KP - 1),
                                    perf_mode=DR)
                        if t % 2 == 0:
                            nc.vector.tensor_scalar(out=kt4d[:, t, 0, :],
                                                    in0=ps, scalar1=IW,
                                                    scalar2=None, op0=OP.mult)
                        else:
                            nc.scalar.activation(out=kt4d[:, t, 0, :], in_=ps,
                                                 func=AF.Copy, scale=IW)

                    def emit_qt(t):
                        ps = pss.tile([128, 256], F32, tag="pss", name="ps_q")
                        for kp in range(KP):
                            nc.tensor.matmul(
                                out=ps,
                                lhsT=wq[:, 2 * kp:2 * kp + 2,
                                        128 * t:128 * (t + 1)],
                                rhs=bT[:, 2 * kp:2 * kp + 2, 0:256],
                                start=(kp == 0), stop=(kp == KP - 1),
                                perf_mode=DR)
                        if t % 2 == 0:
                            nc.vector.tensor_scalar(out=qt4d[:, t, 0, :],
                                                    in0=ps,
                                                    scalar1=IW,
                                                    scalar2=qb[:, t:t + 1],
                                                    op0=OP.mult, op1=OP.add)
                        else:
                            nc.scalar.activation(out=qt4d[:, t, 0, :], in_=ps,
                                                 func=AF.Identity,
                                                 bias=qb[:, t:t + 1],
                                                 scale=IW)


                    # ---------------- attention ----------------
                    xT = own.tile([128, 8, 256], F8, tag="xT", name="xT")
                    # software-pipelined: scores/exp/mult of pair t+1 emitted
                    # BEFORE P@V of pair t so the PE never drains.
                    p_at = {}

                    def emit_scores(t):
                        # z rides the PSUM: scores accumulate q.k, then an
                        # ident8 matmul adds z^T straight from SBUF — exp on
                        # ACT is then the ONLY op between PE score and PE PV.
                        p3s = []
                        for hb in range(2):
                            p3 = pp.tile([128, 8, 256], F8, tag="p3", bufs=4,
                                         name=f"p3_{hb}")
                            zb_t = ezp.tile([128, 8, 256], F8, tag="ez",
                                            bufs=3)
                            nc.sync.dma_start(out=zb_t, in_=vez(t)[:, hb])
                            base = 64 * hb
                            for kh in range(2):   # halves of 4 kt chunks
                                ps_s = psb.tile([128, NF], F32, tag="psb",
                                                name="ps_s")
                                # psum zero-regions are 2KB (512 f32): the
                                # two 256-col quarters in a bank share one
                                # start/stop accumulation group
                                for k4 in range(4):
                                    kt = 4 * kh + k4
                                    nc.tensor.matmul(
                                        out=ps_s[:, 256 * k4:256 * (k4 + 1)],
                                        lhsT=kt4d[base:base + 48, t, :,
                                                  128 * kt:128 * (kt + 1)],
                                        rhs=qt4d[base:base + 48, t, :, :],
                                        start=(k4 % 2 == 0), stop=False,
                                        perf_mode=DR,
                                        skip_group_check=True)
                                    nc.tensor.matmul(
                                        out=ps_s[:, 256 * k4:256 * (k4 + 1)],
                                        lhsT=ident8,
                                        rhs=zb_t[:, kt, :],
                                        start=False, stop=(k4 % 2 == 1),
                                        skip_group_check=True)
                                # ONE exp per [128,1024] (ACT reads may cross
                                # psum banks; matmul writes must not)
                                dst = p3[:, 4 * kh:4 * kh + 4, :] \
                                    .rearrange("p a b -> p (a b)")
                                nc.scalar.activation(out=dst, in_=ps_s,
                                                     func=AF.Exp)
                            p3s.append(p3)
                        p_at[t] = p3s

                    def emit_pv(t):
                        p3s = p_at.pop(t)
                        ps_pv = pss.tile([128, 256], F32, tag="pss",
                                         name="ps_pv")
                        # DoubleRow rejects dst partition offsets (ISA check
                        # s3d3_mm_valid_dst_partition): head hb=0 runs DR at
                        # partition 0, hb=1 falls back to plain fp8 matmuls
                        # (1 cycle/row) at tile_position (0, 64).
                        for kp in range(4):
                            nc.tensor.matmul(
                                out=ps_pv[0:64, :],
                                lhsT=v3d[:, 2 * kp:2 * kp + 2,
                                         HDP * 2 * t:HDP * (2 * t + 1)],
                                rhs=p3s[0][:, 2 * kp:2 * kp + 2, :],
                                start=(kp == 0), stop=False,
                                perf_mode=DR,
                                skip_group_check=True)
                        h = 2 * t + 1
                        for kt in range(8):
                            nc.tensor.matmul(
                                out=ps_pv[64:128, :],
                                lhsT=v3d[:, kt, HDP * h:HDP * (h + 1)],
                                rhs=p3s[1][:, kt, :],
                                start=False, stop=(kt == 7),
                                tile_position=(0, 64),
                                skip_group_check=True)
                        # row sums sit at partitions 0 / 64 (ones cols of v).
                        # ONE op over the contiguous [0:65] partition range
                        # covers both rows (rows 1-63 are live o-values:
                        # nonzero, so their reciprocals stay finite and ind2's
                        # zero weights ignore them). recip(o/16) = 16/o folds
                        # xT's 16x scale.
                        tsum = tmp.tile([128, 256], F32, tag="tsum", bufs=2)
                        nc.vector.tensor_scalar(out=tsum[0:65, :],
                                                in0=ps_pv[0:65, :],
                                                scalar1=IW, scalar2=None,
                                                op0=OP.mult)
                        # recip ONLY the two denominator rows: other rows
                        # include exact-zero padding dims (recip -> inf ->
                        # 0*inf NaN in the broadcast matmul)
                        with nc.allow_low_precision(
                                reason="1/denom to bf16: denom O(1e2+)"):
                            nc.vector.reciprocal(out=rb[0:1, :],
                                                 in_=tsum[0:1, :])
                            nc.vector.reciprocal(out=rb[64:65, :],
                                                 in_=tsum[64:65, :])
                        ps_bc = pss.tile([128, 256], F32, tag="pss",
                                         name="ps_bc")
                        nc.tensor.matmul(out=ps_bc, lhsT=ind2, rhs=rb,
                                         start=True, stop=True)
                        xg = tmp.tile([128, 256], BF16, tag="xg2")
                        nc.vector.tensor_mul(xg, ps_pv, gate_g[t])
                        nc.vector.tensor_mul(xT[:, t, :], xg, ps_bc)

                    # AdaLN2's sb2 path needs only snT: run its GEMMs inside
                    # the ACT-bound attention window (PE has slack there),
                    # staging results in SBUF via Pool (no ACT involvement,
                    # so no Exp-table thrash).
                    sb2S = own.tile([128, FC, 256], F8, tag="b2S",
                                    name="b2S")

                    sig2S = own.tile([128, FC, 256], F32, tag="s2S",
                                     name="s2S")

                    def emit_sb2_pre():
                        wsb2 = load_w8("w_sb2", FC, "w_sb2", eng=nc.gpsimd)
                        for ob in range(FC):
                            ps = pss.tile([128, 256], F32, tag="pss",
                                          name="ps_b2")
                            for kp in range(KP):
                                nc.tensor.matmul(
                                    out=ps,
                                    lhsT=wsb2[:, 2 * kp:2 * kp + 2,
                                              128 * ob:128 * (ob + 1)],
                                    rhs=snT[:, 2 * kp:2 * kp + 2, 0:256],
                                    start=(kp == 0), stop=(kp == KP - 1),
                                    perf_mode=DR)
                            nc.vector.tensor_copy(out=sb2S[:, ob, :], in_=ps)

                    def emit_sig2_pre():
                        # sig2 GEMMs during attention; raw f32 staging so the
                        # ACT sigmoid (table swap!) waits until after the exps
                        wss2 = load_w8("w_ss2", FC, "w_ss2", eng=nc.gpsimd)
                        for ob in range(FC):
                            ps = pss.tile([128, 256], F32, tag="pss",
                                          name="ps_s2")
                            for kp in range(KP):
                                nc.tensor.matmul(
                                    out=ps,
                                    lhsT=wss2[:, 2 * kp:2 * kp + 2,
                                              128 * ob:128 * (ob + 1)],
                                    rhs=snT[:, 2 * kp:2 * kp + 2, 0:256],
                                    start=(kp == 0), stop=(kp == KP - 1),
                                    perf_mode=DR)
                            nc.vector.tensor_scalar(
                                out=sig2S[:, ob, :], in0=ps,
                                scalar1=IW, scalar2=ssb2f[:, ob:ob + 1],
                                op0=OP.mult, op1=OP.add)

                    for t in range(8):
                        emit_kt(t)
                    v3d = kvp.tile([128, RT, HP], F8, tag="v3d", name="v3d")
                    for rt in range(RT):
                        ps = psb.tile([128, HP], F32, tag="psb", name="ps_v")
                        for kp in range(KP):
                            for cs in (0, 512):
                                nc.tensor.matmul(
                                    out=ps[:, cs:cs + 512],
                                    lhsT=bT[:, 2 * kp:2 * kp + 2,
                                            128 * rt:128 * (rt + 1)],
                                    rhs=wv[:, 2 * kp:2 * kp + 2, cs:cs + 512],
                                    start=(kp == 0), stop=(kp == KP - 1),
                                    perf_mode=DR)
                        if rt % 2 == 0:
                            nc.scalar.activation(out=v3d[:, rt, :], in_=ps,
                                                 func=AF.Copy, scale=IW)
                        else:
                            nc.vector.tensor_scalar(out=v3d[:, rt, :], in0=ps,
                                                    scalar1=IW, scalar2=None,
                                                    op0=OP.mult)
                    # ones columns (rowsum rides PV): col 0 of each 64-block
                    nc.gpsimd.memset(
                        v3d.rearrange("p r (h d) -> p r h d", d=HDP)[:, :, :, 0],
                        1.0)
                    for t in range(8):
                        emit_qt(t)
                    # gates first: ACT sigmoids must precede the exp stream
                    wg = load_w8("w_g", FC, "wg")
                    gate_g = []
                    for t in range(8):
                        ps = pss.tile([128, 256], F32, tag="pss", name="ps_g")
                        for kp in range(KP):
                            nc.tensor.matmul(
                                out=ps,
                                lhsT=wg[:, 2 * kp:2 * kp + 2,
                                        128 * t:128 * (t + 1)],
                                rhs=bT[:, 2 * kp:2 * kp + 2, 0:256],
                                start=(kp == 0), stop=(kp == KP - 1),
                                perf_mode=DR)
                        gt = own.tile([128, 256], BF16, tag=f"gt{t}",
                                      name=f"gt{t}")
                        nc.scalar.activation(out=gt, in_=ps, func=AF.Sigmoid,
                                             scale=IW)
                        gate_g.append(gt)

                    for t in range(8):  # head pairs
                        emit_scores(t)
                        if t > 0:
                            emit_pv(t - 1)
                        if t == 2 and stage >= 4:
                            emit_sb2_pre()
                        if t == 4 and stage >= 4:
                            emit_sig2_pre()
                    emit_pv(7)
                    wow = load_w8("w_ow", 8, "wow")

                    # output projection: x = xT.T @ o_w  (fp8 DR, 4 kpairs)
                    ps_x = [psb.tile([128, D], F32, tag="psb", name="ps_x")
                            for _ in range(2)]
                    for kp in range(4):
                        for rt in range(2):
                            for cs in (0, 512):
                                ce = min(cs + 512, D)
                                nc.tensor.matmul(
                                    out=ps_x[rt][:, cs:ce],
                                    lhsT=xT[:, 2 * kp:2 * kp + 2,
                                            128 * rt:128 * (rt + 1)],
                                    rhs=wow[:, 2 * kp:2 * kp + 2, cs:ce],
                                    start=(kp == 0), stop=(kp == 3),
                                    perf_mode=DR)

                    a1_sb = []
                    for rt in range(2):
                        xg = tmp.tile([128, D], BF16, tag="xg")
                        nc.vector.scalar_tensor_tensor(
                            out=xg, in0=ps_x[rt], scalar=IW2, in1=og_sb[rt],
                            op0=OP.mult, op1=OP.mult)
                        a1 = own.tile([128, D], BF16, tag=f"a1_{rt}",
                                      name=f"a1_{rt}")
                        nc.gpsimd.tensor_add(a1, a_own[rt], xg)
                        a1_sb.append(a1)

                    if stage == 3:
                        for rt in range(2):
                            yt = tmp.tile([128, D], F32, tag="yt", bufs=2)
                            nc.vector.tensor_copy(out=yt, in_=a1_sb[rt])
                            nc.sync.dma_start(
                                out=y_out.ap()[128 * rt:128 * (rt + 1), :],
                                in_=yt)
                    if stage >= 4:
                        # AdaLN2: GEMMs already ran during attention; only
                        # the sigmoid (from staged SBUF f32) remains here.
                        an2 = [own.tile([128, D], BF16, tag=f"an2_{rt}",
                                        name=f"an2_{rt}") for rt in range(2)]
                        ln_pair(a1_sb[0], a1_sb[1], "an2a", "an2b", dsts=an2)
                        sig2T = own.tile([128, 8, 256], F8, tag="xT",
                                         name="s2T")
                        nc.scalar.activation(
                            out=sig2T[:, 0:FC, :].rearrange(
                                "p a b -> p (a b)"),
                            in_=sig2S.rearrange("p a b -> p (a b)"),
                            func=AF.Sigmoid)
                        # a2T: transpose an2, then mul by sig2T and add the
                        # sb2S staged during attention
                        a2T = own.tile([128, FC, 256], F8, tag="a2T",
                                       name="a2T")
                        transpose_group(a2T, an2, 0)
                        for ob in range(FC):
                            nc.gpsimd.tensor_mul(a2T[:, ob, :], a2T[:, ob, :],
                                                 sig2T[:, ob, :])
                            nc.gpsimd.tensor_add(a2T[:, ob, :], a2T[:, ob, :],
                                                 sb2S[:, ob, :])

                        # ------------- transition (feature-on-partition) ----
                        def proj_convB_dr(w3d):
                            """12 octs of [128, 256] DR projections."""
                            outs = []
                            for t in range(12):
                                ps = pss.tile([128, 256], F32, tag="pss",
                                              name="ps_cb")
                                for kp in range(KP):
                                    nc.tensor.matmul(
                                        out=ps,
                                        lhsT=w3d[:, 2 * kp:2 * kp + 2,
                                                 128 * t:128 * (t + 1)],
                                        rhs=a2T[:, 2 * kp:2 * kp + 2, :],
                                        start=(kp == 0), stop=(kp == KP - 1),
                                        perf_mode=DR)
                                outs.append(ps)
                            return outs

                        wsu = load_w8("w_swu", FC, "wsu")
                        u_sb = []
                        for t, ps in enumerate(proj_convB_dr(wsu)):
                            ut = nat.tile([128, 256], BF16, tag="nat768",
                                          bufs=18, name=f"u{t}")
                            if t % 2 == 0:
                                nc.vector.tensor_scalar(
                                    out=ut, in0=ps, scalar1=IW, scalar2=None,
                                    op0=OP.mult)
                            else:
                                nc.scalar.activation(out=ut, in_=ps,
                                                     func=AF.Copy, scale=IW)
                            u_sb.append(ut)
                        wsg = load_w8("w_swg", FC, "wsg", eng=nc.gpsimd)
                        sg_sb = []
                        for t, ps in enumerate(proj_convB_dr(wsg)):
                            st_ = nat.tile([128, 256], BF16, tag="nat768",
                                           bufs=18, name=f"sg{t}")
                            nc.scalar.activation(out=st_, in_=ps, func=AF.Silu,
                                                 scale=IW)
                            sg_sb.append(st_)

                        wba = wp.tile([128, 12, D], F8, tag="wslot", bufs=5,
                                      name="wba")
                        vv = w8in.ap()[_OFFSETS8["w_ba"]:
                                       _OFFSETS8["w_ba"] + HID * D]
                        nc.sync.dma_start(
                            out=wba,
                            in_=vv.rearrange("(c p f) -> p c f", p=128, f=D))
                        wab = load_w8("w_ab", FC, "wab", eng=nc.gpsimd)
                        hT = own.tile([128, 12, 256], F8, tag="hT", name="hT")
                        for t, ps in enumerate(proj_convB_dr(wab)):
                            hu = tmp.tile([128, 256], BF16, tag="hu")
                            nc.gpsimd.tensor_mul(hu, sg_sb[t], u_sb[t])
                            # hT = hu * (16 x a2@ab) -> 16x true scale in fp8
                            nc.vector.tensor_mul(hT[:, t, :], hu, ps)
                        ps_t = [psb.tile([128, D], F32, tag="psb", name="ps_t")
                                for _ in range(2)]
                        for kp in range(6):
                            for rt in range(2):
                                for cs in (0, 512):
                                    ce = min(cs + 512, D)
                                    nc.tensor.matmul(
                                        out=ps_t[rt][:, cs:ce],
                                        lhsT=hT[:, 2 * kp:2 * kp + 2,
                                                128 * rt:128 * (rt + 1)],
                                        rhs=wba[:, 2 * kp:2 * kp + 2, cs:ce],
                                        start=(kp == 0), stop=(kp == 5),
                                        perf_mode=DR)

                        for rt in range(2):
                            for hi, (hs, he) in enumerate(((0, 384), (384, D))):
                                tg = tmp.tile([128, 384], BF16, tag="tg")
                                nc.vector.scalar_tensor_tensor(
                                    out=tg, in0=ps_t[rt][:, hs:he],
                                    scalar=IW2, in1=opg_sb[rt][:, hs:he],
                                    op0=OP.mult, op1=OP.mult)
                                yt = tmp.tile([128, 384], F32, tag="yt",
                                              bufs=2)
                                nc.gpsimd.tensor_add(
                                    yt, a1_sb[rt][:, hs:he], tg)
                                e = nc.sync if hi == 0 else nc.gpsimd
                                e.dma_start(
                                    out=y_out.ap()[128 * rt:128 * (rt + 1),
                                                   hs:he],
                                    in_=yt)

    nc.finalize()
    return nc


def _get_nc():
    if "nc" not in _NC_CACHE:
        _NC_CACHE["nc"] = _build_nc()
    return _NC_CACHE["nc"]


def _pad_cols(w):
    """[768, 768] -> [768, 1024]: each head's 48 cols at a 64-aligned block."""
    wp = np.zeros((D, HP), np.float32)
    wp.reshape(D, H, HDP)[:, :, :HD] = np.asarray(w, np.float32).reshape(D, H, HD)
    return wp


def _bf(x):
    return np.ascontiguousarray(np.asarray(x, np.float32).astype(NPBF16))


def _f8(x):
    return np.ascontiguousarray(
        (np.asarray(x, np.float32) * WSC).astype(NPF8))


def _f8u(x):
    """fp8 at TRUE scale: for weights whose PSUM is consumed without a
    descale hook (direct adds into bT / a2T)."""
    return np.ascontiguousarray(np.asarray(x, np.float32).astype(NPF8))


def kernel(**inputs):
    a = np.asarray(inputs["a"], np.float32)
    s = np.asarray(inputs["s"], np.float32)
    z = np.asarray(inputs["z"], np.float32)

    snw1 = np.asarray(inputs["adaln1_snw"], np.float32)[:, None]
    snw2 = np.asarray(inputs["adaln2_snw"], np.float32)[:, None]
    w_ss1 = _f8(snw1 * np.asarray(inputs["adaln1_ssw"], np.float32))
    ssb1 = _bf(np.asarray(inputs["adaln1_ssb"], np.float32).reshape(6, 128))
    w_sb1 = _f8u(snw1 * np.asarray(inputs["adaln1_sbw"], np.float32))
    w_ss2 = _f8(snw2 * np.asarray(inputs["adaln2_ssw"], np.float32))
    ssb2 = _bf(np.asarray(inputs["adaln2_ssb"], np.float32).reshape(6, 128))
    w_sb2 = _f8u(snw2 * np.asarray(inputs["adaln2_sbw"], np.float32))

    w_q = _f8(_pad_cols(inputs["q_w"]) * SCALE)
    qb_p = np.zeros((H, HDP), np.float32)
    qb_p[:, :HD] = np.asarray(inputs["q_b"], np.float32).reshape(H, HD) * SCALE
    qb_p = np.ascontiguousarray(qb_p.reshape(8, 128))
    w_k = _f8(_pad_cols(inputs["k_w"]))
    w_g = _f8(_pad_cols(inputs["g_w"]))
    w_vp = np.zeros((D, HP), np.float32)
    w_vp.reshape(D, H, HDP)[:, :, 1:HD + 1] = \
        np.asarray(inputs["v_w"], np.float32).reshape(D, H, HD)
    w_v = _f8(w_vp)
    w_ow = np.zeros((HP, D), np.float32)
    w_ow.reshape(H, HDP, D)[:, 1:HD + 1, :] = \
        np.asarray(inputs["o_w"], np.float32).reshape(H, HD, D)
    w_ow = _f8(w_ow)
    w_op1 = _f8(inputs["outproj_w"])
    bop1 = _bf(np.asarray(inputs["outproj_b"], np.float32)[None, :])
    w_op2 = _f8(inputs["op_w"])
    bop2 = _bf(np.asarray(inputs["op_b"], np.float32)[None, :])
    sw = np.asarray(inputs["swish_w"], np.float32)
    w_swu = _f8(sw[:, :HID])
    w_swg = _f8(sw[:, HID:])
    w_ab = _f8(inputs["a2b_w"])
    w_ba = _f8(inputs["b2a_w"])

    ident = _bf(np.eye(128))
    ind2 = np.zeros((128, 128), np.float32)
    ind2[0, 0:64] = 1.0
    ind2[64, 64:128] = 1.0
    ind2 = _bf(ind2)

    shared_bf = dict(
        ident_in=ident, ind2_in=ind2, qb_in=_bf(qb_p),
        ssb1_in=ssb1, ssb2_in=ssb2, bop1_in=bop1, bop2_in=bop2,
    )
    shared_tail_bf = np.concatenate(
        [np.asarray(shared_bf[nm], NPBF16).reshape(-1)
         for nm, _ in _LAYOUT[2:]])

    shared_w8 = dict(
        w_ss1=w_ss1, w_sb1=w_sb1, w_k=w_k, w_v=w_v, w_q=w_q, w_g=w_g,
        w_ow=w_ow, w_ss2=w_ss2, w_sb2=w_sb2, w_op1=w_op1, w_op2=w_op2,
        w_swu=w_swu, w_swg=w_swg, w_ab=w_ab, w_ba=w_ba,
        ident8_in=np.eye(128, dtype=np.float32).astype(NPF8),
    )
    shared_tail_w8 = np.concatenate(
        [np.asarray(shared_w8[nm], NPF8).reshape(-1)
         for nm, _ in _LAYOUT8[1:]])

    sbf = s.astype(NPBF16)
    abf = a.astype(NPBF16)

    in_maps = []
    for c in range(8):
        beta, q0 = c // 4, 256 * (c % 4)
        perm = np.r_[q0:q0 + 256, 0:q0, q0 + 256:1024]
        ez = z[:, beta, q0:q0 + 256, :]              # [16, 256q, 1024k] (raw z)
        ez = ez[:, :, perm]                          # rotate k to own-first
        ez = ez.transpose(0, 2, 1)                   # [16, 1024k, 256q]
        ez = ez.reshape(2 * 8, 8, 128, R)            # [(t hb), kt, p, q]
        ez = ez.reshape(8, 2, 8, 128, R).transpose(0, 3, 1, 2, 4)
        blob = np.concatenate([
            sbf[beta][perm].reshape(-1),
            abf[beta][perm].reshape(-1),
            shared_tail_bf,
        ])
        blob8 = np.concatenate([
            np.ascontiguousarray(ez.astype(NPF8)).reshape(-1),
            shared_tail_w8,
        ])
        in_maps.append({"xin": blob, "w8in": blob8})

    nc = _get_nc()
    global _LAST_IN_MAPS
    _LAST_IN_MAPS = in_maps
    if os.environ.get("KERNEL_PREP_ONLY"):
        return np.zeros((B, N, D), np.float32)
    res = run_bass_kernel_spmd(nc, in_maps, core_ids=list(range(8)))

    out = np.empty((B, N, D), np.float32)
    for c in range(8):
        beta, q0 = c // 4, 256 * (c % 4)
        out[beta, q0:q0 + 256, :] = res.results[c]["y"]
    return out
